# revision 1
# baseline (speedup 1.0000x reference)
"""Cross-attention reducer kernel for Trainium2, 8 NeuronCores (SPMD).

Problem (full shapes):
    token_input    [T=8192, L=4096]
    learned_queries[V=4096, I=512]
    w_q [I, I], w_k [L, I], w_v [L, I], w_out [I, L]

    q = learned_queries @ w_q;  k = token_input @ w_k;  v = token_input @ w_v
    per head h (H=8, D=64): attn = softmax(q_h k_h^T / sqrt(D)); out_h = attn @ v_h
    out = concat_h(out_h) @ w_out      -> [V, L]

Sharding: queries (V) are sharded 8 ways; the K/V projections are
sequence-parallel (each core projects its T/8 token shard with the full
w_k/w_v) followed by an AllGather of k^T and v, after which every core runs
attention for all 8 heads over its own 512 queries and its full-T gathered
k/v, then applies the output projection for its V-shard. The final output is
computed transposed (final^T = w_out^T-contraction) so every matmul contracts
on the partition dimension with no large transposes anywhere:

    q^T  [I, Vs]  = w_q (lhsT)  x lq^T (rhs)
    k^T  [I, t]   = w_k (lhsT)  x tok^T (rhs)        (per T-shard, gathered)
    v^T  [I, t]   = w_v (lhsT)  x tok^T (rhs), then 128x128 PE-transposes
                    to v [t, I] before the gather
    s^T  [t, Vs]  = k_h^T (lhsT) x q_h^T (rhs)       (t-tiles of 128)
    p^T           = exp(s^T / 8)                      (no max-subtraction:
                    scores are O(3), exp can't overflow; identical math)
    u^T  [D+1,Vs] = [v_h | 1] (lhsT) x p^T (rhs)     (row D = softmax denom)
    a^T  [D, Vs]  = u^T * (1/denom broadcast via PE outer product)
    out^T[L, Vs]  = w_out (lhsT) x a^T (rhs)

Matmul dtypes: fp32r (full-rate fp32, ~6.5e-5 rel err) for all projections
and the output projection; bf16 for the gathered k/v + attention matmuls
(halves gather traffic; ~0.5% worst-case contribution, well under tolerance).
"""

import os

import numpy as np

import concourse.bacc as bacc
import concourse.tile as tile
import concourse.mybir as mybir
from concourse.bass_utils import run_bass_kernel_spmd

F32 = mybir.dt.float32
F32R = mybir.dt.float32r
BF16 = mybir.dt.bfloat16
EXP = mybir.ActivationFunctionType.Exp
EQ = mybir.AluOpType.is_equal

N_CORES = 8
T, L, V, INNER = 8192, 4096, 4096, 512
H, D = 8, 64
TS = T // N_CORES      # 1024  t-shard per core
QS = V // N_CORES      # 512   query shard per core
SCALE = D ** -0.5      # 0.125

NT = T // 128          # 64 t-tiles per head
GRP = 3                # t-tiles per exp batch (3 psum banks)

# diagnostics: BASSK_F32=1 -> plain f32 matmuls + sync-engine loads (no casts)
_USE_F32 = bool(os.environ.get("BASSK_F32"))
_PHASES = os.environ.get("BASSK_PHASES", "all")  # all | proj | attn


def build_program():
    FR = F32 if _USE_F32 else F32R
    ld = None  # set after nc exists
    nc = bacc.Bacc(
        "TRN2", target_bir_lowering=False, debug=False, num_devices=N_CORES
    )

    tok_T = nc.dram_tensor("tok_T", [L, TS], F32, kind="ExternalInput").ap()
    lq_T = nc.dram_tensor("lq_T", [INNER, QS], F32, kind="ExternalInput").ap()
    w_q = nc.dram_tensor("w_q", [INNER, INNER], F32, kind="ExternalInput").ap()
    w_k = nc.dram_tensor("w_k", [L, INNER], F32, kind="ExternalInput").ap()
    w_v = nc.dram_tensor("w_v", [L, INNER], F32, kind="ExternalInput").ap()
    w_out = nc.dram_tensor("w_out", [INNER, L], F32, kind="ExternalInput").ap()
    outT = nc.dram_tensor("outT", [L, QS], F32, kind="ExternalOutput").ap()

    # rearranged DRAM views (partition-major for SBUF loads)
    tok_T_v = tok_T.rearrange("(k p) t -> p k t", p=128)        # [128, 32, 1024]
    lq_v = lq_T.rearrange("(k p) q -> p k q", p=128)            # [128, 4, 512]
    w_q_v = w_q.rearrange("(k p) i -> p k i", p=128)            # [128, 4, 512]
    w_k_v = w_k.rearrange("(k p) i -> p k i", p=128)            # [128, 32, 512]
    w_v_v = w_v.rearrange("(k p) i -> p k i", p=128)            # [128, 32, 512]
    w_out_v = w_out.rearrange("(k p) l -> p k l", p=128)        # [128, 4, 4096]

    ld_eng = (lambda: nc.sync) if _USE_F32 else (lambda: nc.gpsimd)

    with tile.TileContext(nc) as tc:
        with (
            tc.tile_pool(name="persist", bufs=1) as persist,
            tc.tile_pool(name="dram", bufs=1, space="DRAM") as dram,
        ):
            # ---- persistent SBUF across phases ----
            qT_sb = persist.tile([64, H, QS], BF16, tag="qT")        # q^T per head
            kT_sb = persist.tile([128, 4, TS], BF16, tag="kT")       # k^T shard
            v_sb = persist.tile([128, TS // 128, INNER], BF16, tag="v")  # v shard
            aT_sb = persist.tile([128, 4, QS], FR, tag="aT")       # attn out^T
            idn = persist.tile([128, 128], BF16, tag="idn")          # identity
            ones_64 = persist.tile([1, D], F32, tag="ones64")

            # collective bounce buffers
            gk_in = dram.tile([INNER, TS], BF16, tag="gk_in")
            gk_out = dram.tile([N_CORES * INNER, TS], BF16, tag="gk_out")
            gv_in = dram.tile([TS, INNER], BF16, tag="gv_in")
            gv_out = dram.tile([N_CORES * TS, INNER], BF16, tag="gv_out")

            # identity matrix for PE transposes: idn[p, f] = (f == p)
            with tc.tile_pool(name="idpool", bufs=1) as idp:
                irow = idp.tile([128, 128], F32, tag="irow")
                icol = idp.tile([128, 1], F32, tag="icol")
                nc.gpsimd.iota(irow[:], pattern=[[1, 128]], base=0, channel_multiplier=0, allow_small_or_imprecise_dtypes=True)
                nc.gpsimd.iota(icol[:], pattern=[[0, 1]], base=0, channel_multiplier=1, allow_small_or_imprecise_dtypes=True)
                nc.vector.tensor_scalar(idn[:], irow[:], icol[:], None, EQ)
            nc.vector.memset(ones_64[:], 1.0)

            # ================= phase 1: projections =================
            if _PHASES in ("all", "proj"):
              with (
                  tc.tile_pool(name="proj", bufs=2) as proj,
                  tc.tile_pool(name="projq", bufs=1) as projq,
                  tc.tile_pool(name="pps", bufs=2, space="PSUM") as pps,
                  tc.tile_pool(name="ppq", bufs=1, space="PSUM") as ppq,
              ):
                  # --- q^T = w_q^T-contraction: lhsT=w_q tile, rhs=lq^T tile
                  wq_sb = projq.tile([128, 4, INNER], FR, tag="wq")
                  lq_sb = projq.tile([128, 4, QS], FR, tag="lq")
                  ld_eng().dma_start(wq_sb[:], w_q_v)
                  ld_eng().dma_start(lq_sb[:], lq_v)
                  for m in range(4):
                      ps = ppq.tile([128, QS], F32, tag="psq")
                      for kk in range(4):
                          nc.tensor.matmul(
                              ps[:],
                              wq_sb[:, kk, m * 128:(m + 1) * 128],
                              lq_sb[:, kk, :],
                              start=(kk == 0),
                              stop=(kk == 3),
                          )
                      qstage = projq.tile([128, QS], BF16, tag="qstage")
                      nc.vector.tensor_copy(qstage[:], ps[:])
                      # shift each head's 64 rows down to base partition 0
                      nc.sync.dma_start(qT_sb[:, 2 * m, :], qstage[0:64, :])
                      nc.sync.dma_start(qT_sb[:, 2 * m + 1, :], qstage[64:128, :])

                  # --- k^T and v^T projections, t streamed in two halves
                  for th in range(2):
                      tok_sb = proj.tile([128, 32, 512], FR, tag="tok", bufs=1)
                      ld_eng().dma_start(tok_sb[:], tok_T_v[:, :, th * 512:(th + 1) * 512])
                      # k^T [i-block m, t-half]
                      for m in range(4):
                          wcol = proj.tile([128, 32, 128], FR, tag="wcol")
                          ld_eng().dma_start(wcol[:], w_k_v[:, :, m * 128:(m + 1) * 128])
                          ps = pps.tile([128, 512], F32, tag="pp")
                          for k in range(32):
                              nc.tensor.matmul(
                                  ps[:], wcol[:, k, :], tok_sb[:, k, :],
                                  start=(k == 0), stop=(k == 31),
                              )
                          nc.vector.tensor_copy(kT_sb[:, m, th * 512:(th + 1) * 512], ps[:])
                      # v^T then transpose to v [t, i]
                      for m in range(4):
                          wcol = proj.tile([128, 32, 128], FR, tag="wcol")
                          ld_eng().dma_start(wcol[:], w_v_v[:, :, m * 128:(m + 1) * 128])
                          ps = pps.tile([128, 512], F32, tag="pp")
                          for k in range(32):
                              nc.tensor.matmul(
                                  ps[:], wcol[:, k, :], tok_sb[:, k, :],
                                  start=(k == 0), stop=(k == 31),
                              )
                          vst = proj.tile([128, 512], BF16, tag="vstage")
                          nc.vector.tensor_copy(vst[:], ps[:])
                          pt = pps.tile([128, 512], BF16, tag="pt")
                          for j in range(4):
                              nc.tensor.transpose(
                                  pt[:, j * 128:(j + 1) * 128],
                                  vst[:, j * 128:(j + 1) * 128],
                                  idn[:],
                              )
                          # pt columns j hold v[t-chunk j of this half, i-block m]
                          nc.vector.tensor_copy(
                              v_sb[:, th * 4:(th + 1) * 4, m * 128:(m + 1) * 128],
                              pt[:].rearrange("p (j i) -> p j i", j=4),
                          )

                  # bounce + gather
                  nc.sync.dma_start(
                      gk_in.rearrange("(m p) t -> p m t", p=128), kT_sb[:]
                  )
                  nc.sync.dma_start(
                      gv_in.rearrange("(j p) i -> p j i", p=128), v_sb[:]
                  )
                  if os.environ.get("BASSK_NO_CC"):
                      # timing-only variant: skip the collectives (wrong data)
                      nc.sync.dma_start(gk_out[0:INNER, :], gk_in[:])
                      nc.sync.dma_start(gv_out[0:TS, :], gv_in[:])
                  else:
                      nc.gpsimd.collective_compute(
                          "AllGather", mybir.AluOpType.bypass,
                          replica_groups=[list(range(N_CORES))],
                          ins=[gk_in.opt()], outs=[gk_out.opt()],
                      )
                      nc.gpsimd.collective_compute(
                          "AllGather", mybir.AluOpType.bypass,
                          replica_groups=[list(range(N_CORES))],
                          ins=[gv_in.opt()], outs=[gv_out.opt()],
                      )

            # ================= phase 2: attention =================
            if _PHASES == "attn":
                nc.vector.memset(qT_sb[:], 0.001)
            gv_v = gv_out.rearrange("(c j p) i -> c p j i", p=128, j=TS // 128)
            groups = [list(range(s, min(s + GRP, NT))) for s in range(0, NT, GRP)]

            if _PHASES in ("all", "attn"):
              with (
                tc.tile_pool(name="attn", bufs=2) as attn,
                tc.tile_pool(name="attn3", bufs=3) as attn3,
                  tc.tile_pool(name="aps", bufs=2, space="PSUM") as aps,
                  tc.tile_pool(name="aps1", bufs=1, space="PSUM") as aps1,
              ):
                  # all heads' V loaded once with contiguous 1KB runs
                  vh_all = attn.tile([128, NT, INNER], BF16, tag="vh_all", bufs=1)
                  for c in range(N_CORES):
                      nc.sync.dma_start(
                          vh_all[:, c * (TS // 128):(c + 1) * (TS // 128), :],
                          gv_v[c, :, :, :],
                      )
                  for h in range(H):
                      kTh = attn.tile([64, N_CORES, TS], BF16, tag="kTh")
                      for c in range(N_CORES):
                          nc.sync.dma_start(
                              kTh[:, c, :],
                              gk_out[c * INNER + h * D: c * INNER + h * D + D, :],
                          )
                      vh = attn.tile([128, NT, D + 1], BF16, tag="vh")
                      nc.vector.memset(vh[:, :, D], 1.0)
                      nc.vector.tensor_copy(
                          vh[:, :, 0:D], vh_all[:, :, h * D:(h + 1) * D]
                      )
                      qTh = qT_sb[:, h, :]

                      ps_o = aps1.tile([D + 1, QS], F32, tag="ps_o")
                      prev = None  # (group, pT tile)
                      for g in groups:
                          ps_s = aps.tile([128, GRP * QS], F32, tag="ps_s")
                          for jj, j in enumerate(g):
                              nc.tensor.matmul(
                                  ps_s[:, jj * QS:(jj + 1) * QS],
                                  kTh[:, j // (TS // 128), (j % (TS // 128)) * 128:
                                      (j % (TS // 128)) * 128 + 128],
                                  qTh,
                                  start=True, stop=True,
                              )
                          pT = attn3.tile([128, GRP * QS], BF16, tag="pT")
                          n = len(g) * QS
                          nc.scalar.activation(pT[:, 0:n], ps_s[:, 0:n], EXP, scale=SCALE)
                          if prev is not None:
                              pg, ppT = prev
                              for jj, j in enumerate(pg):
                                  nc.tensor.matmul(
                                      ps_o[:], vh[:, j, :], ppT[:, jj * QS:(jj + 1) * QS],
                                      start=(j == 0), stop=(j == NT - 1),
                                      skip_group_check=True,
                                  )
                          prev = (g, pT)
                      pg, ppT = prev
                      for jj, j in enumerate(pg):
                          nc.tensor.matmul(
                              ps_o[:], vh[:, j, :], ppT[:, jj * QS:(jj + 1) * QS],
                              start=(j == 0), stop=(j == NT - 1),
                              skip_group_check=True,
                          )

                      # normalize: a^T = u^T / denom  (denom broadcast via PE)
                      u_sb = attn.tile([D + 1, QS], F32, tag="u")
                      nc.vector.tensor_copy(u_sb[:], ps_o[:])
                      dn0 = attn.tile([1, QS], F32, tag="dn0")
                      nc.sync.dma_start(dn0[:], u_sb[D:D + 1, :])  # shift to partition 0
                      recip = attn.tile([1, QS], F32, tag="recip")
                      nc.vector.reciprocal(recip[:], dn0[:])
                      ps_r = aps.tile([D, QS], F32, tag="ps_s")  # borrow a ps_s slot
                      nc.tensor.matmul(ps_r[:], ones_64[:], recip[:], start=True, stop=True)
                      a_tmp = attn.tile([D, QS], F32, tag="a_tmp")
                      nc.vector.tensor_mul(a_tmp[:], u_sb[0:D, :], ps_r[:])
                      ld_eng().dma_start(
                          aT_sb[(h % 2) * 64:(h % 2) * 64 + 64, h // 2, :], a_tmp[:]
                      )

            # ================= phase 3: output projection =================
            if _PHASES in ("all", "attn"):
              with (
                tc.tile_pool(name="outp", bufs=3) as outp,
                tc.tile_pool(name="ops", bufs=2, space="PSUM") as ops,
              ):
                  for m in range(L // 128):
                      wo = outp.tile([128, 4, 128], FR, tag="wo")
                      ld_eng().dma_start(wo[:], w_out_v[:, :, m * 128:(m + 1) * 128])
                      ps = ops.tile([128, QS], F32, tag="po")
                      for kk in range(4):
                          nc.tensor.matmul(
                              ps[:], wo[:, kk, :], aT_sb[:, kk, :],
                              start=(kk == 0), stop=(kk == 3),
                          )
                      of = outp.tile([128, QS], F32, tag="of")
                      nc.vector.tensor_copy(of[:], ps[:])
                      nc.sync.dma_start(outT[m * 128:(m + 1) * 128, :], of[:])

    nc.compile()
    return nc


_COMPILED = None


def _get_compiled():
    global _COMPILED
    if _COMPILED is None:
        _COMPILED = build_program()
    return _COMPILED


def make_in_maps(token_input, learned_queries, w_q, w_k, w_v, w_out):
    token_input = np.ascontiguousarray(np.asarray(token_input, dtype=np.float32))
    learned_queries = np.ascontiguousarray(np.asarray(learned_queries, dtype=np.float32))
    w_q = np.ascontiguousarray(np.asarray(w_q, dtype=np.float32))
    w_k = np.ascontiguousarray(np.asarray(w_k, dtype=np.float32))
    w_v = np.ascontiguousarray(np.asarray(w_v, dtype=np.float32))
    w_out = np.ascontiguousarray(np.asarray(w_out, dtype=np.float32))
    in_maps = []
    for c in range(N_CORES):
        in_maps.append({
            "tok_T": np.ascontiguousarray(token_input[c * TS:(c + 1) * TS, :].T),
            "lq_T": np.ascontiguousarray(learned_queries[c * QS:(c + 1) * QS, :].T),
            "w_q": w_q, "w_k": w_k, "w_v": w_v, "w_out": w_out,
        })
    return in_maps


def assemble(results):
    out = np.empty((V, L), dtype=np.float32)
    for c in range(N_CORES):
        out[c * QS:(c + 1) * QS, :] = results[c]["outT"].T
    return out


def kernel(token_input, learned_queries, w_q, w_k, w_v, w_out):
    nc = _get_compiled()
    in_maps = make_in_maps(token_input, learned_queries, w_q, w_k, w_v, w_out)
    res = run_bass_kernel_spmd(nc, in_maps, list(range(N_CORES)))
    return assemble(res.results)



# revision 9
# speedup vs baseline: 1.2120x; 1.2120x over previous
"""Cross-attention reducer kernel for Trainium2, 8 NeuronCores (SPMD).

Problem (full shapes):
    token_input    [T=8192, L=4096]
    learned_queries[V=4096, I=512]
    w_q [I, I], w_k [L, I], w_v [L, I], w_out [I, L]

    q = learned_queries @ w_q;  k = token_input @ w_k;  v = token_input @ w_v
    per head h (H=8, D=64): attn = softmax(q_h k_h^T / sqrt(D)); out_h = attn @ v_h
    out = concat_h(out_h) @ w_out      -> [V, L]

Sharding: sequence-parallel attention with an all-gather of q and per-head
ReduceScatters of partial softmax sums (flash-style partial-sum reduction):

  - each core projects its T/8 token shard to local k, v with the full
    w_k/w_v (no k/v communication at all) and its V/8 query shard to q.
  - q (bf16, 0.5 MiB/core) is all-gathered early; the gather hides under
    the k/v projections.
  - every core computes, for ALL 4096 queries x 8 heads, the partial
    numerator u = sum_{t in shard} exp(s_t) v_t and the partial denominator
    sum_t exp(s_t) over its local t shard only (exp without max-subtraction:
    logits are O(3), overflow impossible; identical math to softmax).
  - per head, the partial [V, D+1] f32 sums are ReduceScatter-summed over
    cores; core r receives the fully-reduced sums of its own V/8 queries.
    The 8 small collectives pipeline behind later heads' attention compute,
    so only the last head's reduction is exposed.
  - normalize (reciprocal of the reduced denominator, a per-partition
    scalar multiply) and apply the output projection for the V/8 shard.

All matmuls keep the contraction on the partition dimension, full depth:
    k^T, v^T [I, t] = w (lhsT) x tok^T (rhs)        contraction L, 128-deep
    s^T [t, q]      = k_h^T (lhsT) x q_h^T (rhs)    contraction D=64
    u   [q, D+1]    = p^T (lhsT) x [v_h | 1] (rhs)  contraction t, 128-deep
    out^T [L, q]    = w_out (lhsT) x a^T (rhs)      contraction I, 128-deep
The PV matmul contracts over t (full 128 PE rows) instead of producing only
D+1=65 output partitions - half the PE column-cycles of a u^T formulation -
and the denominator rides along as a ones column appended to every v head
block. Transposes (v^T -> v, a -> a^T) use the DMA XBAR, not the PE.

Engine queues: PE matmuls; ACT small loads + partition shifts + XBAR
transposes + all exp; DVE psum drains + normalize; SP token loads, q tiles,
u bounces, output stores; Pool (gpsimd) exclusively collectives + the one
casting load of w_out (software DGE casts f32->bf16 in the DMA).
"""

import os

import numpy as np

import concourse.bacc as bacc
import concourse.tile as tile
import concourse.mybir as mybir
from concourse.bass_utils import run_bass_kernel_spmd

F32 = mybir.dt.float32
F32R = mybir.dt.float32r
BF16 = mybir.dt.bfloat16
EXP = mybir.ActivationFunctionType.Exp
MULT = mybir.AluOpType.mult
ADD = mybir.AluOpType.add
EQ = mybir.AluOpType.is_equal

N_CORES = 8
T, L, V, INNER = 8192, 4096, 4096, 512
H, D = 8, 64
TS = T // N_CORES      # 1024  t-shard per core
QS = V // N_CORES      # 512   query shard per core (= ReduceScatter block)
NTS = TS // 128        # 8     local t-tiles
SCALE = D ** -0.5      # 0.125
VW = D + 1             # 65    v head block width incl. ones column
GROUPS = ((0, 1, 2), (3, 4, 5), (6, 7))   # t-tile exp batches (3 psum banks)

# diagnostics: BASSK_NO_CC=1 -> replace collectives with local copies
# (wrong data, timing only)
_NO_CC = bool(os.environ.get("BASSK_NO_CC"))


def build_program():
    nc = bacc.Bacc(
        "TRN2", target_bir_lowering=False, debug=False, num_devices=N_CORES
    )

    # f32r == f32 bit-identical; declaring DRAM as f32r lets any engine load
    # it without the gpsimd cast path while the PE runs full-rate fp32.
    tok_T = nc.dram_tensor("tok_T", [L, TS], F32R, kind="ExternalInput").ap()
    lq_T = nc.dram_tensor("lq_T", [INNER, QS], F32R, kind="ExternalInput").ap()
    w_q = nc.dram_tensor("w_q", [INNER, INNER], F32R, kind="ExternalInput").ap()
    w_k = nc.dram_tensor("w_k", [L, INNER], F32R, kind="ExternalInput").ap()
    w_v = nc.dram_tensor("w_v", [L, INNER], F32R, kind="ExternalInput").ap()
    w_out = nc.dram_tensor("w_out", [INNER, L], F32, kind="ExternalInput").ap()
    outT = nc.dram_tensor("outT", [L, QS], F32, kind="ExternalOutput").ap()

    # partition-major DRAM views
    tok_v = tok_T.rearrange("(k p) t -> p k t", p=128)      # [128, 32, 1024]
    lq_v = lq_T.rearrange("(k p) q -> p k q", p=128)        # [128, 4, 512]
    w_q_v = w_q.rearrange("(k p) i -> p k i", p=128)        # [128, 4, 512]
    w_k_v = w_k.rearrange("(k p) i -> p k i", p=128)        # [128, 32, 512]
    w_v_v = w_v.rearrange("(k p) i -> p k i", p=128)        # [128, 32, 512]
    w_out_v = w_out.rearrange("(k p) l -> p k l", p=128)    # [128, 4, 4096]

    with tile.TileContext(nc) as tc:
        with (
            tc.tile_pool(name="persist", bufs=1) as persist,
            tc.tile_pool(name="dram", bufs=1, space="DRAM") as dram,
        ):
            # ---- persistent SBUF ----
            kTh_loc = persist.tile([64, H, TS], BF16, tag="kTh")     # local k^T per head
            v_sb = persist.tile([128, NTS, H * VW], BF16, tag="v")   # local v + ones cols
            aT_sb = persist.tile([128, 4, QS], BF16, tag="aT")       # normalized attn out^T
            idn = persist.tile([128, 128], BF16, tag="idn")          # identity (PE transpose)

            # collective buffers
            gq_in = dram.tile([INNER, QS], BF16, tag="gq_in")
            gq_out = dram.tile([N_CORES * INNER, QS], BF16, tag="gq_out")
            u_dram = dram.tile([H, V, VW], F32, tag="u_dram")
            u_red = dram.tile([H, QS, VW], F32, tag="u_red")

            gq_view = gq_out.rearrange("(c hh p) q -> p hh c q", p=64, hh=H)

            # identity matrix for PE transposes: idn[p, f] = (f == p)
            with tc.tile_pool(name="idpool", bufs=1) as idp:
                irow = idp.tile([128, 128], F32, tag="irow")
                icol = idp.tile([128, 1], F32, tag="icol")
                nc.gpsimd.iota(irow[:], pattern=[[1, 128]], base=0,
                               channel_multiplier=0,
                               allow_small_or_imprecise_dtypes=True)
                nc.gpsimd.iota(icol[:], pattern=[[0, 1]], base=0,
                               channel_multiplier=1,
                               allow_small_or_imprecise_dtypes=True)
                nc.vector.tensor_scalar(idn[:], irow[:], icol[:], None, EQ)

            # ones columns of v (denominator accumulators)
            for h in range(H):
                nc.vector.memset(v_sb[:, :, h * VW + D], 1.0)

            with (
                tc.tile_pool(name="psS", bufs=2, space="PSUM") as psS,
                tc.tile_pool(name="psA", bufs=2, space="PSUM") as psA,
            ):
                # ================ q projection + gather ================
                with tc.tile_pool(name="qp", bufs=1) as qp:
                    wq_sb = qp.tile([128, 4, INNER], F32R, tag="wq")
                    lq_sb = qp.tile([128, 4, QS], F32R, tag="lq")
                    nc.scalar.dma_start(wq_sb[:], w_q_v)
                    nc.scalar.dma_start(lq_sb[:], lq_v)
                    for m in range(4):
                        ps = psA.tile([128, QS], F32, tag="ps")
                        for kk in range(4):
                            nc.tensor.matmul(
                                ps[:],
                                wq_sb[:, kk, m * 128:(m + 1) * 128],
                                lq_sb[:, kk, :],
                                start=(kk == 0), stop=(kk == 3),
                            )
                        qst = qp.tile([128, QS], BF16, tag="qst", bufs=2)
                        nc.vector.tensor_copy(qst[:], ps[:])
                        nc.scalar.dma_start(gq_in[m * 128:(m + 1) * 128, :], qst[:])
                    if _NO_CC:
                        nc.sync.dma_start(gq_out[0:INNER, :], gq_in[:])
                    else:
                        nc.gpsimd.collective_compute(
                            "AllGather", mybir.AluOpType.bypass,
                            replica_groups=[list(range(N_CORES))],
                            ins=[gq_in.opt()], outs=[gq_out.opt()],
                        )

                # ================ k/v projections ================
                with (
                    tc.tile_pool(name="proj", bufs=1) as proj,
                    tc.tile_pool(name="wpool", bufs=2) as wpool,
                    tc.tile_pool(name="stage", bufs=8) as stage,
                ):
                    tok0 = proj.tile([128, 32, 512], F32R, tag="tok0")
                    tok1 = proj.tile([128, 32, 512], F32R, tag="tok1")
                    nc.sync.dma_start(tok0[:], tok_v[:, :, 0:512])
                    nc.sync.dma_start(tok1[:], tok_v[:, :, 512:1024])

                    for m in range(4):
                        wcol = wpool.tile([128, 32, 128], F32R, tag="wcol")
                        nc.scalar.dma_start(wcol[:], w_k_v[:, :, m * 128:(m + 1) * 128])
                        for th, tok in ((0, tok0), (1, tok1)):
                            ps = psA.tile([128, 512], F32, tag="ps")
                            for kk in range(32):
                                nc.tensor.matmul(
                                    ps[:], wcol[:, kk, :], tok[:, kk, :],
                                    start=(kk == 0), stop=(kk == 31),
                                )
                            kst = stage.tile([128, 512], BF16, tag="kst")
                            nc.vector.tensor_copy(kst[:], ps[:])
                            # shift each head's 64 rows down to partition 0
                            sl = slice(th * 512, (th + 1) * 512)
                            nc.scalar.dma_start(kTh_loc[:, 2 * m, sl], kst[0:64, :])
                            nc.scalar.dma_start(
                                kTh_loc[:, 2 * m + 1, sl], kst[64:128, :]
                            )

                    for m in range(4):
                        wcol = wpool.tile([128, 32, 128], F32R, tag="wcol")
                        nc.scalar.dma_start(wcol[:], w_v_v[:, :, m * 128:(m + 1) * 128])
                        for th, tok in ((0, tok0), (1, tok1)):
                            ps = psA.tile([128, 512], F32, tag="ps")
                            for kk in range(32):
                                nc.tensor.matmul(
                                    ps[:], wcol[:, kk, :], tok[:, kk, :],
                                    start=(kk == 0), stop=(kk == 31),
                                )
                            vst = stage.tile([128, 512], BF16, tag="vst")
                            nc.vector.tensor_copy(vst[:], ps[:])
                            # v^T [i, t] -> v [t, i] via PE transpose
                            pt = psS.tile([128, 512], BF16, tag="ss")
                            for j in range(4):
                                nc.tensor.transpose(
                                    pt[:, j * 128:(j + 1) * 128],
                                    vst[:, j * 128:(j + 1) * 128],
                                    idn[:],
                                )
                            # pt[t, (j hh dd)] -> v_sb[t, th*4+j, (2m+hh)*VW + dd]
                            dst = v_sb[
                                :, th * 4:(th + 1) * 4, 2 * m * VW:(2 * m + 2) * VW
                            ].rearrange("p j (hh w) -> p j hh w", hh=2)[:, :, :, 0:D]
                            nc.vector.tensor_copy(
                                dst,
                                pt[:].rearrange("p (j hh w) -> p j hh w", j=4, hh=2),
                            )

                # ================ attention (local t shard, all queries) ====
                with tc.tile_pool(name="attn", bufs=2) as qa:
                    wo_sb = qa.tile([128, 4, L], BF16, tag="wo", bufs=1)

                    def attn_head(h):
                        qTh = qa.tile([64, N_CORES, QS], BF16, tag="qTh")
                        nc.sync.dma_start(qTh[:], gq_view[:, h, :, :])
                        ucb = qa.tile([128, N_CORES, 4, VW], F32, tag="ucb")
                        vh = v_sb[:, :, h * VW:(h + 1) * VW]
                        for c in range(N_CORES):
                            acc = psA.tile([128, 4, VW], F32, tag="ps")

                            def pv(g, pT):
                                for jj, j in enumerate(g):
                                    for qq in range(4):
                                        nc.tensor.matmul(
                                            acc[:, qq, :],
                                            pT[:, jj * QS + qq * 128:
                                               jj * QS + (qq + 1) * 128],
                                            vh[:, j, :],
                                            start=(j == 0 and qq == 0),
                                            stop=(j == NTS - 1 and qq == 3),
                                            skip_group_check=True,
                                        )

                            prev = None
                            for g in GROUPS:
                                ps_s = psS.tile([128, 3 * QS], F32, tag="ss")
                                for jj, j in enumerate(g):
                                    nc.tensor.matmul(
                                        ps_s[:, jj * QS:(jj + 1) * QS],
                                        kTh_loc[:, h, j * 128:(j + 1) * 128],
                                        qTh[:, c, :],
                                        start=True, stop=True,
                                    )
                                pT = qa.tile([128, 3 * QS], BF16, tag="pT", bufs=4)
                                n = len(g) * QS
                                nc.scalar.activation(
                                    pT[:, 0:n], ps_s[:, 0:n], EXP, scale=SCALE
                                )
                                if prev is not None:
                                    pv(*prev)
                                prev = (g, pT)
                            pv(*prev)
                            nc.vector.tensor_copy(ucb[:, c, :, :], acc[:])
                        nc.sync.dma_start(
                            u_dram[h].rearrange("(c qq p) w -> p c qq w", p=128, qq=4),
                            ucb[:],
                        )
                        if _NO_CC:
                            nc.sync.dma_start(u_red[h], u_dram[h, 0:QS, :])
                        else:
                            nc.gpsimd.collective_compute(
                                "ReduceScatter", ADD,
                                replica_groups=[list(range(N_CORES))],
                                ins=[u_dram[h].opt()], outs=[u_red[h].opt()],
                            )

                    for h in range(H):
                        attn_head(h)
                        if h == 1:   # w_out bf16 cast-load between ReduceScatters
                            nc.gpsimd.dma_start(wo_sb[:, :, 0:2048], w_out_v[:, :, 0:2048])
                        if h == 3:
                            nc.gpsimd.dma_start(wo_sb[:, :, 2048:L], w_out_v[:, :, 2048:L])

                    # ---------------- finalize heads ----------------
                    for h in range(H):
                        fin = qa.tile([128, 4, VW], F32, tag="fin")
                        nc.sync.dma_start(
                            fin[:], u_red[h].rearrange("(qq p) w -> p qq w", p=128)
                        )
                        rec = qa.tile([128, 4], F32, tag="rec")
                        nc.vector.reciprocal(rec[:], fin[:, :, D])
                        an = qa.tile([128, 4, D], BF16, tag="an")
                        for qq in range(4):
                            nc.vector.tensor_scalar(
                                an[:, qq, :], fin[:, qq, 0:D], rec[:, qq:qq + 1],
                                None, MULT,
                            )
                        # a [q, d] -> a^T [d, q] via PE transpose
                        pt_a = psS.tile([64, 512], BF16, tag="ss")
                        for qq in range(4):
                            nc.tensor.transpose(
                                pt_a[:, qq * 128:(qq + 1) * 128],
                                an[:, qq, :],
                                idn[:],
                            )
                        nc.vector.tensor_copy(
                            aT_sb[(h % 2) * 64:(h % 2) * 64 + 64, h // 2, :],
                            pt_a[:],
                        )

                    # ================ output projection ================
                    with tc.tile_pool(name="outp", bufs=2) as outp:
                        for mo in range(L // 128):
                            ps = psA.tile([128, QS], F32, tag="ps")
                            for kk in range(4):
                                nc.tensor.matmul(
                                    ps[:], wo_sb[:, kk, mo * 128:(mo + 1) * 128],
                                    aT_sb[:, kk, :],
                                    start=(kk == 0), stop=(kk == 3),
                                )
                            of = outp.tile([128, QS], F32, tag="of")
                            nc.vector.tensor_copy(of[:], ps[:])
                            nc.sync.dma_start(outT[mo * 128:(mo + 1) * 128, :], of[:])

    nc.compile()
    return nc


_COMPILED = None


def _get_compiled():
    global _COMPILED
    if _COMPILED is None:
        _COMPILED = build_program()
    return _COMPILED


def make_in_maps(token_input, learned_queries, w_q, w_k, w_v, w_out):
    token_input = np.ascontiguousarray(np.asarray(token_input, dtype=np.float32))
    learned_queries = np.ascontiguousarray(np.asarray(learned_queries, dtype=np.float32))
    w_q = np.ascontiguousarray(np.asarray(w_q, dtype=np.float32))
    w_k = np.ascontiguousarray(np.asarray(w_k, dtype=np.float32))
    w_v = np.ascontiguousarray(np.asarray(w_v, dtype=np.float32))
    w_out = np.ascontiguousarray(np.asarray(w_out, dtype=np.float32))
    in_maps = []
    for c in range(N_CORES):
        in_maps.append({
            "tok_T": np.ascontiguousarray(token_input[c * TS:(c + 1) * TS, :].T),
            "lq_T": np.ascontiguousarray(learned_queries[c * QS:(c + 1) * QS, :].T),
            "w_q": w_q, "w_k": w_k, "w_v": w_v, "w_out": w_out,
        })
    return in_maps


def assemble(results):
    out = np.empty((V, L), dtype=np.float32)
    for c in range(N_CORES):
        out[c * QS:(c + 1) * QS, :] = results[c]["outT"].T
    return out


def kernel(token_input, learned_queries, w_q, w_k, w_v, w_out):
    nc = _get_compiled()
    in_maps = make_in_maps(token_input, learned_queries, w_q, w_k, w_v, w_out)
    res = run_bass_kernel_spmd(nc, in_maps, list(range(N_CORES)))
    return assemble(res.results)


# revision 12
# speedup vs baseline: 1.4734x; 1.2157x over previous
"""Cross-attention reducer kernel for Trainium2, 8 NeuronCores (SPMD).

Problem (full shapes):
    token_input    [T=8192, L=4096]
    learned_queries[V=4096, I=512]
    w_q [I, I], w_k [L, I], w_v [L, I], w_out [I, L]

    q = learned_queries @ w_q;  k = token_input @ w_k;  v = token_input @ w_v
    per head h (H=8, D=64): attn = softmax(q_h k_h^T / sqrt(D)); out_h = attn @ v_h
    out = concat_h(out_h) @ w_out      -> [V, L]

Sharding: sequence-parallel attention with an all-gather of q and per-head
ReduceScatters of partial softmax sums (flash-style partial-sum reduction):

  - each core projects its T/8 token shard to local k, v with the full
    w_k/w_v (no k/v communication at all) and its V/8 query shard to q.
  - q (bf16, 0.5 MiB/core) is all-gathered early; the gather hides under
    the k projection.
  - every core computes, for ALL 4096 queries x 8 heads, the partial
    numerator u = sum_{t in shard} exp(s_t) v_t and the partial denominator
    sum_t exp(s_t) over its local t shard only (exp without max-subtraction:
    logits are O(3), overflow impossible; identical math to softmax).
  - per head, the partial [V, D+1] f32 sums are ReduceScatter-summed over
    cores; core r receives the fully-reduced sums of its own V/8 queries.
    The 8 small collectives pipeline behind later heads' attention compute,
    so only the last head's reduction is exposed.
  - normalize (reciprocal of the reduced denominator, a per-partition
    scalar multiply) and apply the output projection for the V/8 shard.

The t shard is processed in two halves, and the projection of each half is
interleaved with its attention pass head-pair by head-pair, so the exp
stream (the critical engine: all exp runs on the one Activation engine)
starts ~55us into the kernel instead of after all projections. The second
half's partial sums are merged into the first's with an accumulate-DMA
(software DGE f32 add) into the u staging buffer in DRAM.

All matmuls keep the contraction on the partition dimension, full depth:
    k^T, v^T [I, t] = w (lhsT) x tok^T (rhs)        contraction L, 128-deep
    s^T [t, q]      = k_h^T (lhsT) x q_h^T (rhs)    contraction D=64
    u   [q, D+1]    = p^T (lhsT) x [v_h | 1] (rhs)  contraction t, 128-deep
    out^T [L, q]    = w_out (lhsT) x a^T (rhs)      contraction I, 128-deep
The PV matmul contracts over t (full 128 PE rows), and the denominator
rides along as a ones column appended to every v head block.

Engine queues: PE matmuls + transposes; ACT first-half weight loads +
k-shifts + all exp; DVE psum drains + normalize; SP token loads, q tiles,
second-half weight loads, u bounces, output stores; Pool (gpsimd)
collectives, the tok second-half load, accumulate-bounces, and the casting
load of w_out.
"""

import os

import numpy as np

import concourse.bacc as bacc
import concourse.tile as tile
import concourse.mybir as mybir
from concourse.bass_utils import run_bass_kernel_spmd

F32 = mybir.dt.float32
F32R = mybir.dt.float32r
BF16 = mybir.dt.bfloat16
EXP = mybir.ActivationFunctionType.Exp
MULT = mybir.AluOpType.mult
ADD = mybir.AluOpType.add
EQ = mybir.AluOpType.is_equal

N_CORES = 8
T, L, V, INNER = 8192, 4096, 4096, 512
H, D = 8, 64
TS = T // N_CORES      # 1024  t-shard per core
QS = V // N_CORES      # 512   query shard per core (= ReduceScatter block)
NTS = TS // 128        # 8     local t-tiles
SCALE = D ** -0.5      # 0.125
VW = D + 1             # 65    v head block width incl. ones column
GROUPS_H = ((0, 1), (2, 3))   # per-half t-tile exp batches

# diagnostics: BASSK_NO_CC=1 -> replace collectives with local copies
# (wrong data, timing only)
_NO_CC = bool(os.environ.get("BASSK_NO_CC"))


def build_program():
    nc = bacc.Bacc(
        "TRN2", target_bir_lowering=False, debug=False, num_devices=N_CORES
    )

    # f32r == f32 bit-identical; declaring DRAM as f32r lets any engine load
    # it without the gpsimd cast path while the PE runs full-rate fp32.
    tok_T = nc.dram_tensor("tok_T", [L, TS], F32R, kind="ExternalInput").ap()
    lq_T = nc.dram_tensor("lq_T", [INNER, QS], F32R, kind="ExternalInput").ap()
    w_q = nc.dram_tensor("w_q", [INNER, INNER], F32R, kind="ExternalInput").ap()
    w_k = nc.dram_tensor("w_k", [L, INNER], F32R, kind="ExternalInput").ap()
    w_v = nc.dram_tensor("w_v", [L, INNER], F32R, kind="ExternalInput").ap()
    w_out = nc.dram_tensor("w_out", [INNER, L], F32, kind="ExternalInput").ap()
    outT = nc.dram_tensor("outT", [L, QS], F32, kind="ExternalOutput").ap()

    # partition-major DRAM views
    tok_v = tok_T.rearrange("(k p) t -> p k t", p=128)      # [128, 32, 1024]
    lq_v = lq_T.rearrange("(k p) q -> p k q", p=128)        # [128, 4, 512]
    w_q_v = w_q.rearrange("(k p) i -> p k i", p=128)        # [128, 4, 512]
    w_k_v = w_k.rearrange("(k p) i -> p k i", p=128)        # [128, 32, 512]
    w_v_v = w_v.rearrange("(k p) i -> p k i", p=128)        # [128, 32, 512]
    w_out_v = w_out.rearrange("(k p) l -> p k l", p=128)    # [128, 4, 4096]

    with tile.TileContext(nc) as tc:
        with (
            tc.tile_pool(name="persist", bufs=1) as persist,
            tc.tile_pool(name="dram", bufs=1, space="DRAM") as dram,
        ):
            # ---- persistent SBUF ----
            kTh_loc = persist.tile([64, H, TS], BF16, tag="kTh")     # local k^T per head
            v_sb = persist.tile([128, NTS, H * VW], BF16, tag="v")   # local v + ones cols
            aT_sb = persist.tile([128, 4, QS], BF16, tag="aT")       # normalized attn out^T
            idn = persist.tile([128, 128], BF16, tag="idn")          # identity (PE transpose)

            # collective buffers
            gq_in = dram.tile([INNER, QS], BF16, tag="gq_in")
            gq_out = dram.tile([N_CORES * INNER, QS], BF16, tag="gq_out")
            u_dram = dram.tile([H, V, VW], F32, tag="u_dram")
            u_red = dram.tile([H, QS, VW], F32, tag="u_red")

            gq_view = gq_out.rearrange("(c hh p) q -> p hh c q", p=64, hh=H)

            # identity matrix for PE transposes: idn[p, f] = (f == p)
            with tc.tile_pool(name="idpool", bufs=1) as idp:
                irow = idp.tile([128, 128], F32, tag="irow")
                icol = idp.tile([128, 1], F32, tag="icol")
                nc.gpsimd.iota(irow[:], pattern=[[1, 128]], base=0,
                               channel_multiplier=0,
                               allow_small_or_imprecise_dtypes=True)
                nc.gpsimd.iota(icol[:], pattern=[[0, 1]], base=0,
                               channel_multiplier=1,
                               allow_small_or_imprecise_dtypes=True)
                nc.vector.tensor_scalar(idn[:], irow[:], icol[:], None, EQ)

            # ones columns of v (denominator accumulators)
            for h in range(H):
                nc.vector.memset(v_sb[:, :, h * VW + D], 1.0)

            with (
                tc.tile_pool(name="psS", bufs=2, space="PSUM") as psS,
                tc.tile_pool(name="psA", bufs=2, space="PSUM") as psA,
                tc.tile_pool(name="tokp", bufs=1) as tokp,
                tc.tile_pool(name="qa", bufs=2) as qa,
            ):
                # ================ q projection + gather ================
                tok0 = tokp.tile([128, 32, 512], F32R, tag="tok")
                nc.sync.dma_start(tok0[:], tok_v[:, :, 0:512])

                with tc.tile_pool(name="qp", bufs=1) as qp:
                    wq_sb = qp.tile([128, 4, INNER], F32R, tag="wq")
                    lq_sb = qp.tile([128, 4, QS], F32R, tag="lq")
                    nc.scalar.dma_start(wq_sb[:], w_q_v)
                    nc.scalar.dma_start(lq_sb[:], lq_v)
                    for m in range(4):
                        ps = psA.tile([128, QS], F32, tag="ps")
                        for kk in range(4):
                            nc.tensor.matmul(
                                ps[:],
                                wq_sb[:, kk, m * 128:(m + 1) * 128],
                                lq_sb[:, kk, :],
                                start=(kk == 0), stop=(kk == 3),
                            )
                        qst = qp.tile([128, QS], BF16, tag="qst", bufs=2)
                        nc.vector.tensor_copy(qst[:], ps[:])
                        nc.scalar.dma_start(gq_in[m * 128:(m + 1) * 128, :], qst[:])
                    if _NO_CC:
                        nc.sync.dma_start(gq_out[0:INNER, :], gq_in[:])
                    else:
                        nc.gpsimd.collective_compute(
                            "AllGather", mybir.AluOpType.bypass,
                            replica_groups=[list(range(N_CORES))],
                            ins=[gq_in.opt()], outs=[gq_out.opt()],
                        )

                # ---------- helpers ----------
                def kproj_m(m, th, tok, wcol, stage):
                    """k^T i-block m (heads 2m, 2m+1), t half th."""
                    ps = psA.tile([128, 512], F32, tag="ps")
                    for kk in range(32):
                        nc.tensor.matmul(
                            ps[:], wcol[:, kk, :], tok[:, kk, :],
                            start=(kk == 0), stop=(kk == 31),
                        )
                    kst = stage.tile([128, 512], BF16, tag="kst")
                    nc.vector.tensor_copy(kst[:], ps[:])
                    # shift each head's 64 rows down to partition 0
                    sl = slice(th * 512, (th + 1) * 512)
                    sh_eng = nc.scalar if th == 0 else nc.sync
                    sh_eng.dma_start(kTh_loc[:, 2 * m, sl], kst[0:64, :])
                    sh_eng.dma_start(kTh_loc[:, 2 * m + 1, sl], kst[64:128, :])

                def vproj_m(m, th, tok, wcol, stage):
                    """v^T i-block m, t half th; transpose to v [t, i] on PE."""
                    ps = psA.tile([128, 512], F32, tag="ps")
                    for kk in range(32):
                        nc.tensor.matmul(
                            ps[:], wcol[:, kk, :], tok[:, kk, :],
                            start=(kk == 0), stop=(kk == 31),
                        )
                    vst = stage.tile([128, 512], BF16, tag="vst")
                    nc.vector.tensor_copy(vst[:], ps[:])
                    pt = psS.tile([128, 512], BF16, tag="ss")
                    for j in range(4):
                        nc.tensor.transpose(
                            pt[:, j * 128:(j + 1) * 128],
                            vst[:, j * 128:(j + 1) * 128],
                            idn[:],
                        )
                    # pt[t, (j hh dd)] -> v_sb[t, th*4+j, (2m+hh)*VW + dd]
                    dst = v_sb[
                        :, th * 4:(th + 1) * 4, 2 * m * VW:(2 * m + 2) * VW
                    ].rearrange("p j (hh w) -> p j hh w", hh=2)[:, :, :, 0:D]
                    nc.vector.tensor_copy(
                        dst,
                        pt[:].rearrange("p (j hh w) -> p j hh w", j=4, hh=2),
                    )

                def load_qTh(h, eng):
                    qTh = qa.tile([64, N_CORES, QS], BF16, tag="qTh")
                    eng.dma_start(qTh[:], gq_view[:, h, :, :])
                    return qTh

                def attn_head_half(h, th, qTh):
                    """Partial attention for head h over t half th, all
                    queries; merge into u_dram and (second half) reduce."""
                    ucb = qa.tile([128, N_CORES, 4, VW], F32, tag="ucb")
                    vh = v_sb[:, :, h * VW:(h + 1) * VW]
                    for c in range(N_CORES):
                        acc = psA.tile([128, 4, VW], F32, tag="ps")

                        def pv(g, pT):
                            for jj, j in enumerate(g):
                                for qq in range(4):
                                    nc.tensor.matmul(
                                        acc[:, qq, :],
                                        pT[:, jj * QS + qq * 128:
                                           jj * QS + (qq + 1) * 128],
                                        vh[:, th * 4 + j, :],
                                        start=(j == 0 and qq == 0),
                                        stop=(j == 3 and qq == 3),
                                        skip_group_check=True,
                                    )

                        prev = None
                        for g in GROUPS_H:
                            ps_s = psS.tile([128, 3 * QS], F32, tag="ss")
                            for jj, j in enumerate(g):
                                jt = th * 4 + j
                                nc.tensor.matmul(
                                    ps_s[:, jj * QS:(jj + 1) * QS],
                                    kTh_loc[:, h, jt * 128:(jt + 1) * 128],
                                    qTh[:, c, :],
                                    start=True, stop=True,
                                )
                            pT = qa.tile([128, 3 * QS], BF16, tag="pT", bufs=4)
                            n = len(g) * QS
                            nc.scalar.activation(
                                pT[:, 0:n], ps_s[:, 0:n], EXP, scale=SCALE
                            )
                            if prev is not None:
                                pv(*prev)
                            prev = (g, pT)
                        pv(*prev)
                        nc.vector.tensor_copy(ucb[:, c, :, :], acc[:])
                    udst = u_dram[h].rearrange("(c qq p) w -> p c qq w", p=128, qq=4)
                    if th == 0:
                        nc.sync.dma_start(udst, ucb[:])
                    else:
                        nc.gpsimd.dma_start(udst, ucb[:], accum_op=ADD)
                        if _NO_CC:
                            nc.sync.dma_start(u_red[h], u_dram[h, 0:QS, :])
                        else:
                            nc.gpsimd.collective_compute(
                                "ReduceScatter", ADD,
                                replica_groups=[list(range(N_CORES))],
                                ins=[u_dram[h].opt()], outs=[u_red[h].opt()],
                            )

                # ---------------- the two t halves, interleaved ----------------
                with (
                    tc.tile_pool(name="wpool", bufs=3) as wpool,
                    tc.tile_pool(name="stage", bufs=4) as stage,
                ):
                    w_views = {"k": w_k_v, "v": w_v_v}

                    def ldw(kind, m, eng):
                        wcol = wpool.tile([128, 32, 128], F32R, tag="wcol")
                        eng.dma_start(wcol[:], w_views[kind][:, :, m * 128:(m + 1) * 128])
                        return wcol

                    # ---- first t half (weight loads on ACT, q tiles on SP) ----
                    wk = ldw("k", 0, nc.scalar)
                    wv = ldw("v", 0, nc.scalar)
                    kproj_m(0, 0, tok0, wk, stage)
                    vproj_m(0, 0, tok0, wv, stage)
                    wk = ldw("k", 1, nc.scalar)
                    wv = ldw("v", 1, nc.scalar)
                    kproj_m(1, 0, tok0, wk, stage)
                    vproj_m(1, 0, tok0, wv, stage)
                    qT0 = load_qTh(0, nc.sync)
                    qT1 = load_qTh(1, nc.sync)
                    wk = ldw("k", 2, nc.scalar)
                    wv = ldw("v", 2, nc.scalar)
                    attn_head_half(0, 0, qT0)
                    attn_head_half(1, 0, qT1)
                    qT2 = load_qTh(2, nc.sync)
                    qT3 = load_qTh(3, nc.sync)
                    kproj_m(2, 0, tok0, wk, stage)
                    vproj_m(2, 0, tok0, wv, stage)
                    wk = ldw("k", 3, nc.scalar)
                    wv = ldw("v", 3, nc.scalar)
                    attn_head_half(2, 0, qT2)
                    attn_head_half(3, 0, qT3)
                    qT4 = load_qTh(4, nc.sync)
                    qT5 = load_qTh(5, nc.sync)
                    kproj_m(3, 0, tok0, wk, stage)
                    vproj_m(3, 0, tok0, wv, stage)
                    # second-half tok: emitted after the last tok0 use; rides
                    # the Pool queue right behind the q all-gather
                    tok1 = tokp.tile([128, 32, 512], F32R, tag="tok")
                    nc.gpsimd.dma_start(tok1[:], tok_v[:, :, 512:1024])
                    wkb0 = ldw("k", 0, nc.sync)
                    wvb0 = ldw("v", 0, nc.sync)
                    attn_head_half(4, 0, qT4)
                    attn_head_half(5, 0, qT5)
                    qT6 = load_qTh(6, nc.sync)
                    qT7 = load_qTh(7, nc.sync)
                    wkb1 = ldw("k", 1, nc.sync)
                    wvb1 = ldw("v", 1, nc.sync)
                    attn_head_half(6, 0, qT6)
                    attn_head_half(7, 0, qT7)

                    # ---- second t half (weight + q loads on SP) ----
                    kproj_m(0, 1, tok1, wkb0, stage)
                    vproj_m(0, 1, tok1, wvb0, stage)
                    kproj_m(1, 1, tok1, wkb1, stage)
                    vproj_m(1, 1, tok1, wvb1, stage)
                    qT0 = load_qTh(0, nc.sync)
                    qT1 = load_qTh(1, nc.sync)
                    wk = ldw("k", 2, nc.sync)
                    wv = ldw("v", 2, nc.sync)
                    attn_head_half(0, 1, qT0)
                    attn_head_half(1, 1, qT1)
                    qT2 = load_qTh(2, nc.sync)
                    qT3 = load_qTh(3, nc.sync)
                    kproj_m(2, 1, tok1, wk, stage)
                    vproj_m(2, 1, tok1, wv, stage)
                    wk = ldw("k", 3, nc.sync)
                    wv = ldw("v", 3, nc.sync)
                    attn_head_half(2, 1, qT2)
                    attn_head_half(3, 1, qT3)
                    qT4 = load_qTh(4, nc.sync)
                    qT5 = load_qTh(5, nc.sync)
                    kproj_m(3, 1, tok1, wk, stage)
                    vproj_m(3, 1, tok1, wv, stage)
                    attn_head_half(4, 1, qT4)
                    attn_head_half(5, 1, qT5)
                    qT6 = load_qTh(6, nc.sync)
                    qT7 = load_qTh(7, nc.sync)
                    attn_head_half(6, 1, qT6)
                    attn_head_half(7, 1, qT7)

                # ---------------- finalize heads + output projection --------
                with tc.tile_pool(name="finp", bufs=2) as fp:
                    for h in range(H):
                        fin = fp.tile([128, 4, VW], F32, tag="fin")
                        nc.sync.dma_start(
                            fin[:], u_red[h].rearrange("(qq p) w -> p qq w", p=128)
                        )
                        rec = fp.tile([128, 4], F32, tag="rec")
                        nc.vector.reciprocal(rec[:], fin[:, :, D])
                        an = fp.tile([128, 4, D], BF16, tag="an")
                        for qq in range(4):
                            nc.vector.tensor_scalar(
                                an[:, qq, :], fin[:, qq, 0:D], rec[:, qq:qq + 1],
                                None, MULT,
                            )
                        # a [q, d] -> a^T [d, q] via PE transpose
                        pt_a = psS.tile([64, 512], BF16, tag="ss")
                        for qq in range(4):
                            nc.tensor.transpose(
                                pt_a[:, qq * 128:(qq + 1) * 128],
                                an[:, qq, :],
                                idn[:],
                            )
                        nc.vector.tensor_copy(
                            aT_sb[(h % 2) * 64:(h % 2) * 64 + 64, h // 2, :],
                            pt_a[:],
                        )

                    # w_out streamed per m-tile as a casting load on Pool
                    with tc.tile_pool(name="outp", bufs=3) as outp:
                        for mo in range(L // 128):
                            wo = outp.tile([128, 4, 128], BF16, tag="wo")
                            nc.gpsimd.dma_start(
                                wo[:], w_out_v[:, :, mo * 128:(mo + 1) * 128]
                            )
                            ps = psA.tile([128, QS], F32, tag="ps")
                            for kk in range(4):
                                nc.tensor.matmul(
                                    ps[:], wo[:, kk, :], aT_sb[:, kk, :],
                                    start=(kk == 0), stop=(kk == 3),
                                )
                            of = outp.tile([128, QS], F32, tag="of")
                            nc.vector.tensor_copy(of[:], ps[:])
                            nc.sync.dma_start(outT[mo * 128:(mo + 1) * 128, :], of[:])

    nc.compile()
    return nc


_COMPILED = None


def _get_compiled():
    global _COMPILED
    if _COMPILED is None:
        _COMPILED = build_program()
    return _COMPILED


def make_in_maps(token_input, learned_queries, w_q, w_k, w_v, w_out):
    token_input = np.ascontiguousarray(np.asarray(token_input, dtype=np.float32))
    learned_queries = np.ascontiguousarray(np.asarray(learned_queries, dtype=np.float32))
    w_q = np.ascontiguousarray(np.asarray(w_q, dtype=np.float32))
    w_k = np.ascontiguousarray(np.asarray(w_k, dtype=np.float32))
    w_v = np.ascontiguousarray(np.asarray(w_v, dtype=np.float32))
    w_out = np.ascontiguousarray(np.asarray(w_out, dtype=np.float32))
    in_maps = []
    for c in range(N_CORES):
        in_maps.append({
            "tok_T": np.ascontiguousarray(token_input[c * TS:(c + 1) * TS, :].T),
            "lq_T": np.ascontiguousarray(learned_queries[c * QS:(c + 1) * QS, :].T),
            "w_q": w_q, "w_k": w_k, "w_v": w_v, "w_out": w_out,
        })
    return in_maps


def assemble(results):
    out = np.empty((V, L), dtype=np.float32)
    for c in range(N_CORES):
        out[c * QS:(c + 1) * QS, :] = results[c]["outT"].T
    return out


def kernel(token_input, learned_queries, w_q, w_k, w_v, w_out):
    nc = _get_compiled()
    in_maps = make_in_maps(token_input, learned_queries, w_q, w_k, w_v, w_out)
    res = run_bass_kernel_spmd(nc, in_maps, list(range(N_CORES)))
    return assemble(res.results)
